# revision 1
# baseline (speedup 1.0000x reference)
"""CTLSTM cell fused kernel for 8 Trainium2 NeuronCores.

Strategy (data-parallel over batch):
  - B=16384 rows sharded 2048/core; weights replicated.
  - Host stages transposed operands so the K contraction dim lands on SBUF
    partitions: xh = [x;ht].T -> [1024, 2048/core], w2 = [Wx;Wh].T ->
    [1024, 3584], both cast to bf16 (PE runs 1 col/cycle and FWL hides the
    weight loads; fp32 would serialize a ~190ns LDWEIGHTS per matmul).
    PSUM accumulation stays fp32.
  - Gate columns are host-permuted to [z, d, i, f, o, i_bar, f_bar] so the
    five sigmoid gates are contiguous: per 128-row subtile ACT runs one
    tanh, one sigmoid(-x) and ONE [128,2560] sigmoid, all in place in a
    contiguous [128,3584] pre-activation mega-tile.
  - bf16 allows N=1024 moving: matmuls compute gate PAIRS into 2-bank
    PSUM tiles; DVE drains each pair with a single fused bias-add.
  - softplus(wd) has no ACT table set; computed as -ln(sigmoid(-wd)).
    sigmoid(-wd) from the main pass is stashed in SBUF; Ln chunks at the
    end are forced (explicit deps) after all main-pass ACT ops so the
    activation table switches exactly once.
"""

import numpy as np
import ml_dtypes

import concourse.bacc as bacc
import concourse.bass as bass
import concourse.mybir as mybir
import concourse.tile as tile
from concourse.tile_rust import add_dep_helper
from concourse.bass_utils import run_bass_kernel_spmd

NCORES = 8
B = 16384
I = 512
H = 512
NG = 7
G = NG * H          # 3584
K2 = I + H          # 1024
P = 128
BS = B // NCORES    # 2048 rows per core
NT = BS // P        # 16 subtiles of 128 rows
SUP = 4             # subtiles per supertile (DMA granularity)
NSUP = NT // SUP

BF16 = mybir.dt.bfloat16
F32 = mybir.dt.float32
AF = mybir.ActivationFunctionType
NPBF16 = ml_dtypes.bfloat16

# gate order in the permuted weight/bias layout (reference order is
# i, f, z, o, d, i_bar, f_bar)
PERM = [2, 4, 0, 1, 3, 5, 6]   # -> z, d, i, f, o, i_bar, f_bar

TRACE = False
LAST_RESULTS = None

_nc_cache = None


def _build():
    nc = bacc.Bacc("TRN2", target_bir_lowering=False, debug=False)

    xh = nc.dram_tensor("xh", [K2, BS], BF16, kind="ExternalInput")
    w2 = nc.dram_tensor("w2", [K2, G], BF16, kind="ExternalInput")
    ct = nc.dram_tensor("ct", [BS, H], F32, kind="ExternalInput")
    bb_d = nc.dram_tensor("bb", [P, G], F32, kind="ExternalInput")

    h_d = nc.dram_tensor("h", [BS, H], F32, kind="ExternalOutput")
    c_d = nc.dram_tensor("c", [BS, H], F32, kind="ExternalOutput")
    cb_d = nc.dram_tensor("cb", [BS, H], F32, kind="ExternalOutput")
    o_d = nc.dram_tensor("o", [BS, H], F32, kind="ExternalOutput")
    dr_d = nc.dram_tensor("dr", [BS, H], F32, kind="ExternalOutput")

    last_sn = None  # final main-pass ACT instruction, gates phase 2

    with tile.TileContext(nc) as tc:
        with (
            tc.tile_pool(name="wp", bufs=1) as wp,
            tc.tile_pool(name="cp", bufs=1) as cp,
            tc.tile_pool(name="sp", bufs=1) as sp,
            tc.tile_pool(name="xp", bufs=2) as xp,
            tc.tile_pool(name="ctp", bufs=4) as ctp,
            tc.tile_pool(name="gp", bufs=2) as gp,
            tc.tile_pool(name="pp", bufs=3, space=bass.MemorySpace.PSUM) as pp,
            tc.tile_pool(name="pps", bufs=2, space=bass.MemorySpace.PSUM) as pps,
        ):
            # resident weights: 8 K-chunks of [128, 3584] bf16
            w_sb = []
            for k in range(8):
                wt = wp.tile([P, G], BF16, tag=f"w{k}")
                nc.sync.dma_start(wt[:], w2[k * P:(k + 1) * P, :])
                w_sb.append(wt)
            # broadcast bias [128, 3584] fp32 (bx+bh, host-staged broadcast)
            bb = cp.tile([P, G], F32, tag="bb")
            nc.sync.dma_start(bb[:], bb_d[:])
            # sigmoid(-wd) stash, one [128, 512] slice per subtile
            stash = sp.tile([P, NT, H], F32, tag="stash")

            for s in range(NSUP):
                xhs = []
                for k in range(8):
                    t_ = xp.tile([P, SUP * P], BF16, tag=f"xh{k}")
                    nc.sync.dma_start(
                        t_[:], xh[k * P:(k + 1) * P, s * SUP * P:(s + 1) * SUP * P]
                    )
                    xhs.append(t_)

                for j in range(SUP):
                    t = s * SUP + j
                    bsl = slice(j * P, (j + 1) * P)
                    rows = slice(t * P, (t + 1) * P)

                    ctj = ctp.tile([P, H], F32, tag="ct")
                    nc.sync.dma_start(ctj[:], ct[rows, :])

                    ga = gp.tile([P, G], F32, tag="ga")

                    # gate pairs (z,d), (i,f), (o,ib) then single (fb); each
                    # pair accumulates in a 2-bank PSUM tile drained by one
                    # fused bias-add
                    for pr in range(3):
                        csl = slice(pr * 2 * H, (pr + 1) * 2 * H)
                        acc = pp.tile([P, 2 * H], F32, tag="accp")
                        for half in range(2):
                            gsl = slice((pr * 2 + half) * H,
                                        (pr * 2 + half + 1) * H)
                            hsl = slice(half * H, (half + 1) * H)
                            for k in range(8):
                                nc.tensor.matmul(
                                    acc[:, hsl], xhs[k][:, bsl], w_sb[k][:, gsl],
                                    start=(k == 0), stop=(k == 7),
                                )
                        nc.vector.tensor_add(ga[:, csl], acc[:], bb[:, csl])
                    csl = slice(6 * H, 7 * H)
                    acc = pps.tile([P, H], F32, tag="accs")
                    for k in range(8):
                        nc.tensor.matmul(
                            acc[:], xhs[k][:, bsl], w_sb[k][:, csl],
                            start=(k == 0), stop=(k == 7),
                        )
                    nc.vector.tensor_add(ga[:, csl], acc[:], bb[:, csl])

                    # permuted gate slices of ga
                    Z = ga[:, 0 * H:1 * H]
                    D = ga[:, 1 * H:2 * H]
                    Ii = ga[:, 2 * H:3 * H]
                    F = ga[:, 3 * H:4 * H]
                    O = ga[:, 4 * H:5 * H]
                    IB = ga[:, 5 * H:6 * H]
                    FB = ga[:, 6 * H:7 * H]

                    nc.scalar.activation(Z, Z, AF.Tanh)
                    nc.scalar.activation(stash[:, t, :], D, AF.Sigmoid,
                                         scale=-1.0)
                    nc.scalar.activation(ga[:, 2 * H:], ga[:, 2 * H:], AF.Sigmoid)

                    nc.sync.dma_start(o_d[rows, :], O)

                    nc.vector.tensor_mul(F, F, ctj[:])    # f*ct
                    nc.vector.tensor_mul(Ii, Ii, Z)       # i*z
                    nc.vector.tensor_add(F, F, Ii)        # c
                    nc.sync.dma_start(c_d[rows, :], F)
                    nc.vector.tensor_mul(IB, IB, Z)       # ib*z
                    last_sn = nc.scalar.activation(Z, F, AF.Tanh)  # tanh(c)
                    nc.vector.tensor_mul(FB, FB, ctj[:])  # fb*ct
                    nc.vector.tensor_add(FB, FB, IB)      # cbar
                    nc.sync.dma_start(cb_d[rows, :], FB)
                    nc.vector.tensor_mul(Z, O, Z)         # h = o*tanh(c)
                    nc.sync.dma_start(h_d[rows, :], Z)

            # phase 2: decay_rate = softplus(wd) = -ln(sigmoid(-wd))
            dr_r = dr_d.rearrange("(n t p) c -> n p t c", t=SUP, p=P)
            for chn in range(NSUP):
                chsl = slice(chn * SUP, (chn + 1) * SUP)
                ln = nc.scalar.activation(stash[:, chsl, :], stash[:, chsl, :],
                                          AF.Ln)
                # keep Ln after every main-pass ACT: one table switch total
                add_dep_helper(ln.ins, last_sn.ins, reason="phase2 after phase1")
                nc.vector.tensor_scalar_mul(stash[:, chsl, :], stash[:, chsl, :],
                                            -1.0)
                nc.sync.dma_start(dr_r[chn], stash[:, chsl, :])

    nc.compile()
    return nc




def kernel(x, ht, ct, Wx, bx, Wh, bh):
    global _nc_cache, LAST_RESULTS
    if _nc_cache is None:
        _nc_cache = _build()
    nc = _nc_cache

    x = np.ascontiguousarray(x, dtype=np.float32)
    ht = np.ascontiguousarray(ht, dtype=np.float32)
    ct = np.ascontiguousarray(ct, dtype=np.float32)

    # host staging: transpose/concat/cast + gate permutation + bias broadcast
    xh_full = np.empty((K2, B), dtype=NPBF16)
    xh_full[:I, :] = x.T.astype(NPBF16)
    xh_full[I:, :] = ht.T.astype(NPBF16)

    WxT = np.asarray(Wx, dtype=np.float32).T   # [512, 3584]
    WhT = np.asarray(Wh, dtype=np.float32).T
    bsum = np.asarray(bx, dtype=np.float32) + np.asarray(bh, dtype=np.float32)
    w2 = np.empty((K2, G), dtype=NPBF16)
    bbp = np.empty(G, dtype=np.float32)
    for n, old in enumerate(PERM):
        dsl = slice(n * H, (n + 1) * H)
        ssl = slice(old * H, (old + 1) * H)
        w2[:I, dsl] = WxT[:, ssl].astype(NPBF16)
        w2[I:, dsl] = WhT[:, ssl].astype(NPBF16)
        bbp[dsl] = bsum[ssl]
    bb = np.ascontiguousarray(np.broadcast_to(bbp[None, :], (P, G)))

    in_maps = []
    for cidx in range(NCORES):
        sl = slice(cidx * BS, (cidx + 1) * BS)
        in_maps.append({
            "xh": np.ascontiguousarray(xh_full[:, sl]),
            "w2": w2,
            "ct": ct[sl],
            "bb": bb,
        })

    res = run_bass_kernel_spmd(nc, in_maps, core_ids=list(range(NCORES)),
                               trace=TRACE)
    LAST_RESULTS = res

    outs = {}
    for name in ("h", "c", "cb", "o", "dr"):
        outs[name] = np.concatenate(
            [res.results[cidx][name] for cidx in range(NCORES)], axis=0
        )
    return outs["h"], outs["c"], outs["cb"], outs["o"], outs["dr"]



# revision 7
# speedup vs baseline: 1.2500x; 1.2500x over previous
"""CTLSTM cell fused kernel for 8 Trainium2 NeuronCores — v2.

Strategy (data-parallel over batch, transposed compute layout):
  - B=16384 rows sharded 2048/core; weights replicated.
  - TRANSPOSED GEMM: weights are the PE stationary operand, batch streams
    as moving data, so the output lands as [gate_partition, batch_free].
    Per (k-chunk, gate-block) stationary [128,128] serves several N=512
    moving matmuls -> few LDWEIGHTS, all FWL-hidden.
  - Gates on partitions means the per-gate bias is a PER-PARTITION vector:
    ACT fuses bias-add + nonlinearity in ONE pass directly from PSUM
    (out = act(psum + bias[p])), eliminating the DVE bias pass entirely.
  - All gate tiles / ct / outputs are bf16: DVE tensor ops hit the 2x_1p
    mode (2 elem/cycle/lane), and output DMA traffic halves.
  - decay_rate = softplus(wd) uses the ACT Softplus table directly.
  - Gate order host-permuted to [i,f,o,ib,fb,d,z]: 20 Sigmoid blocks, 4
    Softplus, 4 Tanh(z), then Tanh(c) -> 3 ACT table loads per slab.
  - Batch processed in 2 slabs of 1024/core; PSUM: [128,1024] accumulators
    (2 banks), pool of 4, gate-blocks swept k-outer in groups of 3 so the
    first slab pipelines against the streaming weight DMAs.
"""

import numpy as np
import ml_dtypes

import concourse.bacc as bacc
import concourse.bass as bass
import concourse.mybir as mybir
import concourse.tile as tile
from concourse.tile_rust import add_dep_helper
from concourse.bass_utils import run_bass_kernel_spmd

NCORES = 8
B = 16384
I = 512
H = 512
NG = 7
G = NG * H          # 3584
K2 = I + H          # 1024
P = 128
BS = B // NCORES    # 2048 batch cols per core
SLAB = 1024         # batch cols per slab
NSLAB = BS // SLAB  # 2
NGB = G // P        # 28 gate-blocks of 128
NHB = H // P        # 4 h-blocks
NK = K2 // P        # 8 contraction chunks

BF16 = mybir.dt.bfloat16
F32 = mybir.dt.float32
AF = mybir.ActivationFunctionType
NPBF16 = ml_dtypes.bfloat16

# new gate order -> reference gate index (reference: i,f,z,o,d,ib,fb)
# processing order: i, f, o, ib, fb (Sigmoid), d (Softplus), z (Tanh)
PERM = [0, 1, 3, 5, 6, 4, 2]
GI_I, GI_F, GI_O, GI_IB, GI_FB, GI_D, GI_Z = range(7)
# d-gate: no Softplus table on TRN2 -> sigmoid(-wd) now (same Sigmoid
# table, scale=-1 with host-negated bias), -ln(.) in a later Ln pass
GATE_FUNC = [AF.Sigmoid] * 6 + [AF.Tanh]

# gate-block sweep groups (PSUM: 3 live accumulators x 2 banks + slack)
GB_GROUPS = [list(range(s, min(s + 3, NGB))) for s in range(0, NGB, 3)]

TRACE = False
LAST_RESULTS = None

_nc_cache = None


def _build():
    nc = bacc.Bacc("TRN2", target_bir_lowering=False, debug=False)

    xh = nc.dram_tensor("xh", [K2, BS], BF16, kind="ExternalInput")
    w2 = nc.dram_tensor("w2", [K2, G], BF16, kind="ExternalInput")
    ctT = nc.dram_tensor("ctT", [H, BS], BF16, kind="ExternalInput")
    bias_d = nc.dram_tensor("bias", [P, NGB], F32, kind="ExternalInput")

    h_d = nc.dram_tensor("h", [H, BS], BF16, kind="ExternalOutput")
    c_d = nc.dram_tensor("c", [H, BS], BF16, kind="ExternalOutput")
    cb_d = nc.dram_tensor("cb", [H, BS], BF16, kind="ExternalOutput")
    o_d = nc.dram_tensor("o", [H, BS], BF16, kind="ExternalOutput")
    dr_d = nc.dram_tensor("dr", [H, BS], BF16, kind="ExternalOutput")

    with tile.TileContext(nc) as tc:
        with (
            tc.tile_pool(name="wp", bufs=1) as wp,
            tc.tile_pool(name="cp", bufs=1) as cp,
            tc.tile_pool(name="xp", bufs=2) as xp,
            tc.tile_pool(name="ctp", bufs=2) as ctp,
            tc.tile_pool(name="gp", bufs=1) as gp,
            tc.tile_pool(name="pp", bufs=4, space=bass.MemorySpace.PSUM) as pp,
        ):
            # weight chunks [128, 3584] bf16, resident; DMA split in column
            # halves interleaved with the first xh slab so gb sweep 0 can
            # start before the whole 7MB weight stream lands.
            WSPLIT = 1792
            w_sb = [wp.tile([P, G], BF16, tag=f"w{k}", name=f"w{k}")
                    for k in range(NK)]
            bb = cp.tile([P, NGB], F32, tag="bb")
            nc.sync.dma_start(bb[:], bias_d[:])

            prev_act = None  # ACT program-order chain (table grouping)

            for s in range(NSLAB):
                ssl = slice(s * SLAB, (s + 1) * SLAB)

                xh_s = []
                for k in range(NK):
                    if s == 0:
                        nc.sync.dma_start(w_sb[k][:, 0:WSPLIT],
                                          w2[k * P:(k + 1) * P, 0:WSPLIT])
                    t_ = xp.tile([P, SLAB], BF16, tag=f"xh{k}")
                    nc.sync.dma_start(t_[:], xh[k * P:(k + 1) * P, ssl])
                    xh_s.append(t_)
                if s == 0:
                    for k in range(NK):
                        nc.sync.dma_start(w_sb[k][:, WSPLIT:G],
                                          w2[k * P:(k + 1) * P, WSPLIT:G])

                cts = []
                for hb in range(NHB):
                    t_ = ctp.tile([P, SLAB], BF16, tag=f"ct{hb}")
                    nc.sync.dma_start(t_[:], ctT[hb * P:(hb + 1) * P, ssl])
                    cts.append(t_)

                # gate tiles for this slab (bf16, reused in-place later)
                ga = [gp.tile([P, SLAB], BF16, tag=f"ga{gb}", name=f"ga{gb}")
                      for gb in range(NGB)]

                # ---- GEMM + fused bias/activation drain ----
                for grp in GB_GROUPS:
                    accs = {gb: pp.tile([P, SLAB], F32, tag="acc", name="acc")
                            for gb in grp}
                    for k in range(NK):
                        for gb in grp:
                            stat = w_sb[k][:, gb * P:(gb + 1) * P]
                            for h2 in range(SLAB // 512):
                                csl = slice(h2 * 512, (h2 + 1) * 512)
                                nc.tensor.matmul(
                                    accs[gb][:, csl], stat, xh_s[k][:, csl],
                                    start=(k == 0), stop=(k == NK - 1),
                                )
                    for gb in grp:
                        g = gb // NHB
                        a = nc.scalar.activation(ga[gb][:], accs[gb][:],
                                                 GATE_FUNC[g],
                                                 scale=-1.0 if g == GI_D else 1.0,
                                                 bias=bb[:, gb:gb + 1])
                        if prev_act is not None:
                            add_dep_helper(a.ins, prev_act.ins,
                                           reason="act order")
                        prev_act = a
                        if g == GI_O:
                            hb = gb % NHB
                            nc.sync.dma_start(
                                o_d[hb * P:(hb + 1) * P, ssl], ga[gb][:])

                # ---- elementwise epilogue (all bf16 on DVE 2x) ----
                for hb in range(NHB):
                    Ii = ga[GI_I * NHB + hb]
                    F = ga[GI_F * NHB + hb]
                    O = ga[GI_O * NHB + hb]
                    IB = ga[GI_IB * NHB + hb]
                    FB = ga[GI_FB * NHB + hb]
                    Z = ga[GI_Z * NHB + hb]
                    ct_ = cts[hb]
                    rsl = slice(hb * P, (hb + 1) * P)

                    nc.vector.tensor_mul(F[:], F[:], ct_[:])    # f*ct
                    nc.vector.tensor_mul(Ii[:], Ii[:], Z[:])    # i*z
                    nc.vector.tensor_add(F[:], F[:], Ii[:])     # c
                    nc.sync.dma_start(c_d[rsl, ssl], F[:])
                    nc.vector.tensor_mul(IB[:], IB[:], Z[:])    # ib*z
                    nc.vector.tensor_mul(FB[:], FB[:], ct_[:])  # fb*ct
                    nc.vector.tensor_add(FB[:], FB[:], IB[:])   # cbar
                    nc.sync.dma_start(cb_d[rsl, ssl], FB[:])
                    a = nc.scalar.activation(Z[:], F[:], AF.Tanh)  # tanh(c)
                    add_dep_helper(a.ins, prev_act.ins, reason="act order")
                    prev_act = a
                    nc.vector.tensor_mul(Z[:], Z[:], O[:])      # h
                    nc.sync.dma_start(h_d[rsl, ssl], Z[:])

                # decay_rate = -ln(sigmoid(-wd)); all Ln after all Tanh so
                # the ACT table switches once
                for hb in range(NHB):
                    S = ga[GI_D * NHB + hb]
                    rsl = slice(hb * P, (hb + 1) * P)
                    a = nc.scalar.activation(S[:], S[:], AF.Ln)
                    add_dep_helper(a.ins, prev_act.ins, reason="act order")
                    prev_act = a
                    nc.vector.tensor_scalar_mul(S[:], S[:], -1.0)
                    nc.sync.dma_start(dr_d[rsl, ssl], S[:])

    nc.compile()
    return nc


def kernel(x, ht, ct, Wx, bx, Wh, bh):
    global _nc_cache, LAST_RESULTS
    if _nc_cache is None:
        _nc_cache = _build()
    nc = _nc_cache

    x = np.ascontiguousarray(x, dtype=np.float32)
    ht = np.ascontiguousarray(ht, dtype=np.float32)
    ct = np.ascontiguousarray(ct, dtype=np.float32)

    # host staging: transpose/concat/cast + gate permutation
    xh_full = np.empty((K2, B), dtype=NPBF16)
    xh_full[:I, :] = x.T.astype(NPBF16)
    xh_full[I:, :] = ht.T.astype(NPBF16)
    ctT_full = np.ascontiguousarray(ct.T.astype(NPBF16))

    WxT = np.asarray(Wx, dtype=np.float32).T   # [512, 3584]
    WhT = np.asarray(Wh, dtype=np.float32).T
    bsum = np.asarray(bx, dtype=np.float32) + np.asarray(bh, dtype=np.float32)
    w2 = np.empty((K2, G), dtype=NPBF16)
    bbp = np.empty(G, dtype=np.float32)
    for n, old in enumerate(PERM):
        dsl = slice(n * H, (n + 1) * H)
        ssl = slice(old * H, (old + 1) * H)
        w2[:I, dsl] = WxT[:, ssl].astype(NPBF16)
        w2[I:, dsl] = WhT[:, ssl].astype(NPBF16)
        # d-gate ACT runs with scale=-1: out = sigmoid(-wd) needs -bias
        bbp[dsl] = -bsum[ssl] if n == GI_D else bsum[ssl]
    bias = np.ascontiguousarray(bbp.reshape(NGB, P).T)  # [128, 28]

    in_maps = []
    for cidx in range(NCORES):
        sl = slice(cidx * BS, (cidx + 1) * BS)
        in_maps.append({
            "xh": np.ascontiguousarray(xh_full[:, sl]),
            "w2": w2,
            "ctT": np.ascontiguousarray(ctT_full[:, sl]),
            "bias": bias,
        })

    res = run_bass_kernel_spmd(nc, in_maps, core_ids=list(range(NCORES)),
                               trace=TRACE)
    LAST_RESULTS = res

    outs = {}
    for name in ("h", "c", "cb", "o", "dr"):
        outs[name] = np.concatenate(
            [np.asarray(res.results[cidx][name]).T.astype(np.float32)
             for cidx in range(NCORES)], axis=0
        )
    return outs["h"], outs["c"], outs["cb"], outs["o"], outs["dr"]


# revision 8
# speedup vs baseline: 1.3776x; 1.1020x over previous
"""CTLSTM cell fused kernel for 8 Trainium2 NeuronCores — v3.

Strategy (data-parallel over batch, transposed compute layout):
  - B=16384 rows sharded 2048/core; weights replicated.
  - TRANSPOSED GEMM: weights are the PE stationary operand, batch streams
    as moving data, so the output lands as [gate_partition, batch_free].
  - Gates on partitions means the per-gate bias is a PER-PARTITION vector:
    ACT fuses bias-add + nonlinearity in ONE pass directly from PSUM
    (out = act(psum + bias[p])), no DVE bias pass.
  - All gate tiles / ct / outputs are bf16: DVE tensor ops hit the 2x_1p
    mode (2 elem/cycle/lane), and output DMA traffic halves.
  - Gate order i, f, z, o, ib, d, fb: z drains early so the c / tanh(c) /
    h epilogue overlaps the remaining gate sweeps on DVE/ACT while the PE
    keeps streaming; fb last leaves only the short cb chain as tail.
  - decay_rate = softplus(wd) has no ACT table: sigmoid(-wd) during the
    main sigmoid run (scale=-1, host-negated bias), then -ln(.) with the
    Ln pass chained before the fb drains.
  - Batch processed in 2 slabs of 1024/core; PSUM: [128,1024] accumulators
    (2 banks), pool of 4, gate-blocks swept k-outer in groups of 3 so the
    first slab pipelines against the streaming weight DMAs.
"""

import numpy as np
import ml_dtypes

import concourse.bacc as bacc
import concourse.bass as bass
import concourse.mybir as mybir
import concourse.tile as tile
from concourse.tile_rust import add_dep_helper
from concourse.bass_utils import run_bass_kernel_spmd

NCORES = 8
B = 16384
I = 512
H = 512
NG = 7
G = NG * H          # 3584
K2 = I + H          # 1024
P = 128
BS = B // NCORES    # 2048 batch cols per core
SLAB = 1024         # batch cols per slab
NSLAB = BS // SLAB  # 2
NGB = G // P        # 28 gate-blocks of 128
NHB = H // P        # 4 h-blocks
NK = K2 // P        # 8 contraction chunks

BF16 = mybir.dt.bfloat16
F32 = mybir.dt.float32
AF = mybir.ActivationFunctionType
NPBF16 = ml_dtypes.bfloat16

# new gate order -> reference gate index (reference: i,f,z,o,d,ib,fb)
PERM = [0, 1, 2, 3, 5, 4, 6]
GI_I, GI_F, GI_Z, GI_O, GI_IB, GI_D, GI_FB = range(7)
GATE_FUNC = [AF.Sigmoid, AF.Sigmoid, AF.Tanh, AF.Sigmoid, AF.Sigmoid,
             AF.Sigmoid, AF.Sigmoid]

# gate-block sweep groups (PSUM: 3 live accumulators x 2 banks + slack)
GB_GROUPS = [list(range(s, min(s + 3, NGB))) for s in range(0, NGB, 3)]

TRACE = False
LAST_RESULTS = None

_nc_cache = None


def _build():
    nc = bacc.Bacc("TRN2", target_bir_lowering=False, debug=False)

    xh = nc.dram_tensor("xh", [K2, BS], BF16, kind="ExternalInput")
    w2 = nc.dram_tensor("w2", [K2, G], BF16, kind="ExternalInput")
    ctT = nc.dram_tensor("ctT", [H, BS], BF16, kind="ExternalInput")
    bias_d = nc.dram_tensor("bias", [P, NGB], F32, kind="ExternalInput")

    h_d = nc.dram_tensor("h", [H, BS], BF16, kind="ExternalOutput")
    c_d = nc.dram_tensor("c", [H, BS], BF16, kind="ExternalOutput")
    cb_d = nc.dram_tensor("cb", [H, BS], BF16, kind="ExternalOutput")
    o_d = nc.dram_tensor("o", [H, BS], BF16, kind="ExternalOutput")
    dr_d = nc.dram_tensor("dr", [H, BS], BF16, kind="ExternalOutput")

    with tile.TileContext(nc) as tc:
        with (
            tc.tile_pool(name="wp", bufs=1) as wp,
            tc.tile_pool(name="cp", bufs=1) as cp,
            tc.tile_pool(name="xp", bufs=2) as xp,
            tc.tile_pool(name="ctp", bufs=2) as ctp,
            tc.tile_pool(name="gp", bufs=1) as gp,
            tc.tile_pool(name="pp", bufs=4, space=bass.MemorySpace.PSUM) as pp,
        ):
            # weight chunks [128, 3584] bf16, resident; the first quarter of
            # each chunk lands ahead of the xh stream so gb sweeps start
            # within ~2us, the rest streams behind.
            WSPLIT = 896
            w_sb = [wp.tile([P, G], BF16, tag=f"w{k}", name=f"w{k}")
                    for k in range(NK)]
            bb = cp.tile([P, NGB], F32, tag="bb")

            prev_act = None  # ACT program-order chain (table grouping)

            def chain(a):
                nonlocal prev_act
                if prev_act is not None:
                    add_dep_helper(a.ins, prev_act.ins, reason="act order")
                prev_act = a

            for s in range(NSLAB):
                ssl = slice(s * SLAB, (s + 1) * SLAB)

                xh_s = []
                for k in range(NK):
                    if s == 0:
                        nc.sync.dma_start(w_sb[k][:, 0:WSPLIT],
                                          w2[k * P:(k + 1) * P, 0:WSPLIT])
                    t_ = xp.tile([P, SLAB], BF16, tag=f"xh{k}")
                    nc.sync.dma_start(t_[:], xh[k * P:(k + 1) * P, ssl])
                    xh_s.append(t_)
                if s == 0:
                    nc.sync.dma_start(bb[:], bias_d[:])
                    for k in range(NK):
                        nc.sync.dma_start(w_sb[k][:, WSPLIT:G],
                                          w2[k * P:(k + 1) * P, WSPLIT:G])

                cts = []
                for hb in range(NHB):
                    t_ = ctp.tile([P, SLAB], BF16, tag=f"ct{hb}")
                    nc.sync.dma_start(t_[:], ctT[hb * P:(hb + 1) * P, ssl])
                    cts.append(t_)

                # gate tiles for this slab (bf16, reused in-place later)
                ga = [gp.tile([P, SLAB], BF16, tag=f"ga{gb}", name=f"ga{gb}")
                      for gb in range(NGB)]

                def T(g, hb):
                    return ga[g * NHB + hb]

                def out_dma(dst, hb, src):
                    nc.sync.dma_start(dst[hb * P:(hb + 1) * P, ssl], src[:])

                # epilogue emitters, run as soon as their gate drains
                def epi_z_done():
                    # c = f*ct + i*z into the f tile; tanh(c) into the i tile
                    for hb in range(NHB):
                        F, Ii, Z = T(GI_F, hb), T(GI_I, hb), T(GI_Z, hb)
                        nc.vector.tensor_mul(F[:], F[:], cts[hb][:])
                        nc.vector.tensor_mul(Ii[:], Ii[:], Z[:])
                        nc.vector.tensor_add(F[:], F[:], Ii[:])
                        out_dma(c_d, hb, F)
                    for hb in range(NHB):
                        chain(nc.scalar.activation(T(GI_I, hb)[:],
                                                   T(GI_F, hb)[:], AF.Tanh))

                def epi_o_done():
                    # h = o * tanh(c) into the tanh(c) (= i) tile
                    for hb in range(NHB):
                        Tc = T(GI_I, hb)
                        nc.vector.tensor_mul(Tc[:], Tc[:], T(GI_O, hb)[:])
                        out_dma(h_d, hb, Tc)

                def epi_d_done():
                    # decay_rate = -ln(sigmoid(-wd)), Ln chained before fb
                    for hb in range(NHB):
                        S = T(GI_D, hb)
                        chain(nc.scalar.activation(S[:], S[:], AF.Ln))
                        nc.vector.tensor_scalar_mul(S[:], S[:], -1.0)
                        out_dma(dr_d, hb, S)

                def epi_fb_done():
                    # cbar = fb*ct + ib*z into the fb tile
                    for hb in range(NHB):
                        FB, IB, Z = T(GI_FB, hb), T(GI_IB, hb), T(GI_Z, hb)
                        nc.vector.tensor_mul(IB[:], IB[:], Z[:])
                        nc.vector.tensor_mul(FB[:], FB[:], cts[hb][:])
                        nc.vector.tensor_add(FB[:], FB[:], IB[:])
                        out_dma(cb_d, hb, FB)

                epilogue = {GI_Z: epi_z_done, GI_O: epi_o_done,
                            GI_D: epi_d_done, GI_FB: epi_fb_done}

                # ---- GEMM + fused bias/activation drain ----
                drained = 0
                for grp in GB_GROUPS:
                    accs = {gb: pp.tile([P, SLAB], F32, tag="acc", name="acc")
                            for gb in grp}
                    for k in range(NK):
                        for gb in grp:
                            stat = w_sb[k][:, gb * P:(gb + 1) * P]
                            for h2 in range(SLAB // 512):
                                csl = slice(h2 * 512, (h2 + 1) * 512)
                                nc.tensor.matmul(
                                    accs[gb][:, csl], stat, xh_s[k][:, csl],
                                    start=(k == 0), stop=(k == NK - 1),
                                )
                    for gb in grp:
                        g = gb // NHB
                        chain(nc.scalar.activation(
                            ga[gb][:], accs[gb][:], GATE_FUNC[g],
                            scale=-1.0 if g == GI_D else 1.0,
                            bias=bb[:, gb:gb + 1]))
                        if g == GI_O:
                            out_dma(o_d, gb % NHB, ga[gb])
                        drained += 1
                        if drained % NHB == 0:
                            fn = epilogue.get(drained // NHB - 1)
                            if fn is not None:
                                fn()

    nc.compile()
    return nc


def kernel(x, ht, ct, Wx, bx, Wh, bh):
    global _nc_cache, LAST_RESULTS
    if _nc_cache is None:
        _nc_cache = _build()
    nc = _nc_cache

    x = np.ascontiguousarray(x, dtype=np.float32)
    ht = np.ascontiguousarray(ht, dtype=np.float32)
    ct = np.ascontiguousarray(ct, dtype=np.float32)

    # host staging: transpose/concat/cast + gate permutation
    xh_full = np.empty((K2, B), dtype=NPBF16)
    xh_full[:I, :] = x.T.astype(NPBF16)
    xh_full[I:, :] = ht.T.astype(NPBF16)
    ctT_full = np.ascontiguousarray(ct.T.astype(NPBF16))

    WxT = np.asarray(Wx, dtype=np.float32).T   # [512, 3584]
    WhT = np.asarray(Wh, dtype=np.float32).T
    bsum = np.asarray(bx, dtype=np.float32) + np.asarray(bh, dtype=np.float32)
    w2 = np.empty((K2, G), dtype=NPBF16)
    bbp = np.empty(G, dtype=np.float32)
    for n, old in enumerate(PERM):
        dsl = slice(n * H, (n + 1) * H)
        ssl = slice(old * H, (old + 1) * H)
        w2[:I, dsl] = WxT[:, ssl].astype(NPBF16)
        w2[I:, dsl] = WhT[:, ssl].astype(NPBF16)
        # d-gate ACT runs with scale=-1: out = sigmoid(-wd) needs -bias
        bbp[dsl] = -bsum[ssl] if n == GI_D else bsum[ssl]
    bias = np.ascontiguousarray(bbp.reshape(NGB, P).T)  # [128, 28]

    in_maps = []
    for cidx in range(NCORES):
        sl = slice(cidx * BS, (cidx + 1) * BS)
        in_maps.append({
            "xh": np.ascontiguousarray(xh_full[:, sl]),
            "w2": w2,
            "ctT": np.ascontiguousarray(ctT_full[:, sl]),
            "bias": bias,
        })

    res = run_bass_kernel_spmd(nc, in_maps, core_ids=list(range(NCORES)),
                               trace=TRACE)
    LAST_RESULTS = res

    outs = {}
    for name in ("h", "c", "cb", "o", "dr"):
        outs[name] = np.concatenate(
            [np.asarray(res.results[cidx][name]).T.astype(np.float32)
             for cidx in range(NCORES)], axis=0
        )
    return outs["h"], outs["c"], outs["cb"], outs["o"], outs["dr"]
